# revision 18
# baseline (speedup 1.0000x reference)
"""Trainium2 Bass kernel for ChunkedLocalSelfAttention.

Module: x[B,C,H,W] -> qkv proj -> 8-head local-window attention (17x17
spatial window) -> out proj -> +residual -> 1x1 conv -> relu.
B,C,H,W = 4,256,48,48; N = 2304 tokens per image; head dim 32.

Sharding: 8 cores = 4 batch images x 2 query-row-halves (24 rows each).
Attention output rows only depend on +-8 image rows, so cores need no
communication; the row halo is covered by a 32-row k/v band.

On-core design (scores kept TRANSPOSED: keys on partitions, queries free):
  - 2D key chunking: key chunks are [8 rows x 16 cols] = 128 keys, so the
    score block for a chunk only spans the queries within the +-8 COLUMN
    window of its strip ([8 q-rows x <=32 q-cols] instead of all 48 cols).
    This cuts exp/mask/score/PV element count ~30% vs row-major chunks.
  - k and v are projected from a chunk-major-permuted copy of x (xcm) so
    each chunk's 128 keys are contiguous (matmul stationary APs must have
    a single free dim); q is projected from a row-major copy.
  - k-bias is dropped entirely (constant-per-query terms cancel in
    softmax); q-bias is added during the PSUM->SBUF cast; v-bias is folded
    into the residual on the host (softmax weights sum to 1).
  - score chunks are processed in PAIRS sharing one PSUM bank per head
    (packed 2D crops), halving Activation-engine op count: exp runs on
    [128, 2 heads, E_pair<=448] per op.
  - binary window mask applied multiplicatively AFTER exp via
    scalar_tensor_tensor (TensorScalarPtr hits the DVE 4x perf mode on
    bf16/SBUF; plain TensorTensor only gets 2x).
  - PV += v_chunk.T @ masked per chunk with [v|ones] packed lhsT (M=64);
    the ones columns replicate each head's softmax denominator.
  - normalize: reciprocal (DVE), 32-partition shift (DMA), multiply on
    GpSimd (idle engine), compact to channel order (DMA).
  - out proj, +residual(+out_b+Wo@bv folded on host), 1x1 conv, relu.
"""

import sys

for _p in ("/opt/trn_rl_repo",):
    if _p not in sys.path:
        sys.path.insert(0, _p)

import math

import ml_dtypes
import numpy as np

B, C, H, W = 4, 256, 48, 48
N = H * W
HEADS, HD, HALF = 8, 32, 8
NCORES = 8
ROWS_HALF = H // 2          # 24 query rows per core
NQ = ROWS_HALF * W          # 1152 queries per core
BAND_ROWS = 32              # k/v row band per core (24 + 8 halo)
NKCM = BAND_ROWS * W        # 1536 band tokens, chunk-major
QT = 384                    # queries per tile (8 image rows)
SCALE = 1.0 / math.sqrt(HD)

# 2D chunk geometry: chunks are [8 rows x 16 cols]; strips at cols 0/16/32.
CLO = (0, 8, 24)            # query-col crop per strip
CHI = (24, 40, 48)
NCS = (24, 32, 24)
ES = tuple(8 * n for n in NCS)   # 192, 256, 192

bf16 = ml_dtypes.bfloat16

_PROG = None


def _chunks(qt):
    """[(chunk_id, strip)] for query tile qt; chunk_id indexes the 12
    [8x16] chunks of the 32-row band (band*3 + strip)."""
    nb = 2 if qt == 0 else 3
    boff = (0, 0, 1)[qt]
    return [((br + boff) * 3 + s, s) for br in range(nb) for s in range(3)]


def _pairs(qt):
    ch = _chunks(qt)
    return [ch[i : i + 2] for i in range(0, len(ch), 2)]


def _deltas(qt):
    return (0, 8) if qt == 0 else (-8, 0, 8)


def _mask_offsets():
    """Column offsets of each (kind, op) pair-mask in the packed table."""
    offs = {}
    col = 0
    for kind, deltas in ((0, (0, 8)), (1, (-8, 0, 8))):
        ch = [(d, s) for d in deltas for s in range(3)]
        grps = [ch[i : i + 2] for i in range(0, len(ch), 2)]
        for op_i, grp in enumerate(grps):
            offs[(kind, op_i)] = col
            col += sum(ES[s] for _, s in grp)
    return offs, col


MOFF, MTOT = _mask_offsets()


def _build_program():
    import concourse.bass as bass
    import concourse.mybir as mybir
    import concourse.tile as tile
    from concourse import bacc
    from concourse.alu_op_type import AluOpType as OP

    f32 = mybir.dt.float32
    bft = mybir.dt.bfloat16
    AF = mybir.ActivationFunctionType

    nc = bacc.Bacc(
        "TRN2", target_bir_lowering=False, debug=False, num_devices=NCORES
    )

    def din(name, shape, dt=bft):
        return nc.dram_tensor(name, shape, dt, kind="ExternalInput").ap()

    xq_d = din("xqT", [C, NQ])            # row-major rows [0,24)
    xcm_d = din("xcm", [C, NKCM])         # chunk-major rows [0,32)
    xres_d = din("xres", [C, NQ], f32)
    wqk_d = din("wqkT", [C, 2 * C])
    wv_d = din("wvT", [C, C])
    wo_d = din("woT", [C, C])
    wc_d = din("wcT", [C, C])
    bq_d = din("bq", [128, 2], f32)
    bc_d = din("bcrep", [128, 2], f32)
    mask_d = din("masks", [128, MTOT])
    out_d = nc.dram_tensor("out", [C, NQ], f32, kind="ExternalOutput").ap()

    # SPMD trick: one program serves both row-halves. The host ships half-1
    # images VERTICALLY FLIPPED (attention is equivariant under a row flip),
    # so every core sees half-0 geometry: query rows [0,24), band [0,32).

    with tile.TileContext(nc) as tc:
        import contextlib

        ctx = contextlib.ExitStack()
        with ctx:
            cpool = ctx.enter_context(tc.tile_pool(name="const", bufs=1))
            qkpool = ctx.enter_context(tc.tile_pool(name="qk", bufs=1))
            vpool = ctx.enter_context(tc.tile_pool(name="v", bufs=1))
            epool = ctx.enter_context(tc.tile_pool(name="exp", bufs=4))
            apool = ctx.enter_context(tc.tile_pool(name="attn", bufs=4))
            rpool = ctx.enter_context(tc.tile_pool(name="recip", bufs=3))
            opool = ctx.enter_context(tc.tile_pool(name="outb", bufs=3))
            psA = ctx.enter_context(
                tc.tile_pool(name="psA", bufs=2, space="PSUM")
            )
            psPP = ctx.enter_context(
                tc.tile_pool(name="psPP", bufs=2, space="PSUM")
            )
            psB = ctx.enter_context(
                tc.tile_pool(name="psB", bufs=2, space="PSUM")
            )

            # ---- constants / inputs to SBUF ----
            xq = [cpool.tile([128, NQ], bft, tag=f"xq{t}", name=f"xq{t}") for t in range(2)]
            xcm = [cpool.tile([128, NKCM], bft, tag=f"xcm{t}", name=f"xcm{t}") for t in range(2)]
            wqk = [cpool.tile([128, 2 * C], bft, tag=f"wqk{t}", name=f"wqk{t}") for t in range(2)]
            wv = [cpool.tile([128, C], bft, tag=f"wv{t}", name=f"wv{t}") for t in range(2)]
            wo = [cpool.tile([128, C], bft, tag=f"wo{t}", name=f"wo{t}") for t in range(2)]
            wc = [cpool.tile([128, C], bft, tag=f"wc{t}", name=f"wc{t}") for t in range(2)]
            bq = cpool.tile([128, 2], f32, tag="bq")
            bcr = cpool.tile([128, 2], f32, tag="bcr")
            zrow = cpool.tile([1, 512], bft, tag="zrow")
            nc.vector.memset(zrow[:], 0.0)
            msk = cpool.tile([128, MTOT], bft, tag="msk")
            xres = [cpool.tile([128, NQ], f32, tag=f"xres{t}", name=f"xres{t}") for t in range(2)]
            # preload the Exp table while DMAs are in flight
            actwarm = cpool.tile([1, 8], bft, tag="actwarm")
            nc.scalar.activation(actwarm[:], zrow[0:1, 0:8], AF.Exp)
            # load order = first-use order: q path, then k/v, masks, tail
            def _ld(t, dst, src):
                nc.sync.dma_start(dst[t][:], src[128 * t : 128 * t + 128, :])
            for t in range(2):
                _ld(t, xq, xq_d)
                _ld(t, wqk, wqk_d)
            nc.sync.dma_start(bq[:], bq_d[:])
            for t in range(2):
                _ld(t, xcm, xcm_d)
                _ld(t, wv, wv_d)
            nc.sync.dma_start(msk[:], mask_d[:])
            for t in range(2):
                _ld(t, wo, wo_d)
                _ld(t, wc, wc_d)
                _ld(t, xres, xres_d)
            nc.sync.dma_start(bcr[:], bc_d[:])

            # ---- phase 1: projections ----
            # q tiles [128ch, 1152 row-major]; k tiles [128ch, 1536 chunk-major]
            qkq = [qkpool.tile([128, NQ], bft, tag=f"qkq{i}", name=f"qkq{i}") for i in range(2)]
            qkk = [qkpool.tile([128, NKCM], bft, tag=f"qkk{i}", name=f"qkk{i}") for i in range(2)]
            # v tiles per chunk: head h cols [64h,64h+32)=v_h, [64h+32,64h+64)=1
            vt = [vpool.tile([128, 8 * 64], bft, tag=f"v{i}", name=f"v{i}") for i in range(12)]
            for i in range(12):
                va = vt[i][:].rearrange("p (h two v) -> p h two v", two=2, v=32)
                nc.gpsimd.memset(va[:, :, 1, :], 1.0)

            def q_proj(qc, tiles=(0, 384, 768)):
                for n0 in tiles:
                    ps = psB.tile([128, 512], f32, tag="ps", name="ps")
                    for cc in range(2):
                        nc.tensor.matmul(
                            ps[:, :QT],
                            lhsT=wqk[cc][:, 128 * qc : 128 * qc + 128],
                            rhs=xq[cc][:, n0 : n0 + QT],
                            start=(cc == 0),
                            stop=(cc == 1),
                        )
                    nc.vector.tensor_scalar_add(
                        qkq[qc][:, n0 : n0 + QT], ps[:, :QT], bq[:, qc : qc + 1]
                    )

            def k_proj(kc, tiles=(0, 512, 1024)):
                for n0 in tiles:
                    ps = psB.tile([128, 512], f32, tag="ps", name="ps")
                    for cc in range(2):
                        nc.tensor.matmul(
                            ps[:],
                            lhsT=wqk[cc][:, 256 + 128 * kc : 256 + 128 * kc + 128],
                            rhs=xcm[cc][:, n0 : n0 + 512],
                            start=(cc == 0),
                            stop=(cc == 1),
                        )
                    nc.vector.tensor_copy(qkk[kc][:, n0 : n0 + 512], ps[:])

            def v_proj(i):
                n0 = 128 * i
                ps = psB.tile([128, 512], f32, tag="ps", name="ps")
                for cc in range(2):
                    nc.tensor.matmul(
                        ps[:, :C],
                        lhsT=xcm[cc][:, n0 : n0 + 128],
                        rhs=wv[cc][:],
                        start=(cc == 0),
                        stop=(cc == 1),
                    )
                va = vt[i][:].rearrange("p (h two v) -> p h two v", two=2, v=32)
                nc.scalar.copy(
                    va[:, :, 0, :],
                    ps[:, :C].rearrange("p (h v) -> p h v", v=32),
                )

            # ---- phase 2: attention ----
            oT = [cpool.tile([128, NQ], bft, tag=f"oT{g}", name=f"oT{g}") for g in range(2)]
            res = [cpool.tile([128, NQ], bft, tag=f"res{t}", name=f"res{t}") for t in range(2)]

            def pair_open():
                # pp rows: [pv_a(0:32)|S_a(32:64)|pv_b(64:96)|S_b(96:128)]
                # zero-matmul opens the bank: clears pending-zero over the
                # full [0:384] so per-chunk 2D-crop PVs can accumulate.
                pp = psPP.tile([128, 512], f32, tag="pp", name="pp")
                nc.tensor.matmul(
                    pp[:, 0:QT],
                    lhsT=zrow[:, 0:128],
                    rhs=zrow[:, 0:QT],
                    start=True,
                    stop=False,
                    skip_group_check=True,
                )
                return pp

            def attn_op(qt, g, op_i, pp):
                q0row = 8 * qt
                kind = 0 if qt == 0 else 1
                ops = _pairs(qt)
                if True:
                    for grp in [ops[op_i]]:
                        sc = psA.tile([128, 1024], f32, tag="sc", name="sc")
                        offs = []
                        off = 0
                        for ck_id, s in grp:
                            ncs = NCS[s]
                            for hh in range(2):
                                h = 2 * g + hh
                                qc, krow = h // 4, 32 * (h % 4)
                                out_v = sc[
                                    :, 512 * hh + off : 512 * hh + off + 8 * ncs
                                ].rearrange("p (r c) -> p r c", c=ncs)
                                rhs = qkq[qc][krow : krow + 32, :].rearrange(
                                    "p (r c) -> p r c", c=48
                                )[:, q0row : q0row + 8, CLO[s] : CHI[s]]
                                nc.tensor.matmul(
                                    out_v,
                                    lhsT=qkk[qc][
                                        krow : krow + 32,
                                        128 * ck_id : 128 * ck_id + 128,
                                    ],
                                    rhs=rhs,
                                    start=True,
                                    stop=True,
                                    tile_position=(krow, 0),
                                )
                            offs.append(off)
                            off += 8 * ncs
                        ep = off
                        ex = epool.tile([128, 1024], bft, tag="ex", name="ex")
                        sc_v = sc[:].rearrange("p (h q) -> p h q", q=512)[
                            :, :, 0:ep
                        ]
                        ex_v = ex[:, 0 : 2 * ep].rearrange(
                            "p (h q) -> p h q", q=ep
                        )
                        nc.scalar.activation(ex_v, sc_v, AF.Exp, scale=SCALE)
                        ma = apool.tile([128, 1024], bft, tag="ma", name="ma")
                        ma_v = ma[:, 0 : 2 * ep].rearrange(
                            "p (h q) -> p h q", q=ep
                        )
                        mk = msk[
                            :, MOFF[(kind, op_i)] : MOFF[(kind, op_i)] + ep
                        ]
                        # TensorTensor gets the DVE 2x_1p mode (bf16, packed);
                        # a few ops go to the otherwise-idle GpSimd engine.
                        mask_eng = nc.vector
                        mask_eng.tensor_mul(
                            ma_v,
                            ex_v,
                            mk[:, None, :].broadcast_to([128, 2, ep]),
                        )
                        last_op = op_i == len(ops) - 1
                        for j, (ck_id, s) in enumerate(grp):
                            ncs = NCS[s]
                            vi = vt[ck_id]
                            for hh in range(2):
                                h = 2 * g + hh
                                out_v = pp[
                                    64 * hh : 64 * hh + 64, 0:QT
                                ].rearrange("p (r c) -> p r c", c=48)[
                                    :, :, CLO[s] : CHI[s]
                                ]
                                nc.tensor.matmul(
                                    out_v,
                                    lhsT=vi[:, 64 * h : 64 * h + 64],
                                    rhs=ma[
                                        :,
                                        ep * hh + offs[j] : ep * hh + offs[j] + 8 * ncs,
                                    ],
                                    start=False,
                                    stop=(
                                        last_op
                                        and j == len(grp) - 1
                                        and hh == 1
                                    ),
                                    skip_group_check=True,
                                    tile_position=(0, 64 * hh),
                                )

            def pair_fin(qt, g, pp):
                if True:
                    # normalize: recip sums, shift down 32 partitions onto pv
                    # lanes, multiply, compact to channel order.
                    rc = rpool.tile([128, QT], f32, tag="rc", name="rc")
                    nc.vector.reciprocal(rc[:], pp[:, 0:QT])
                    rcs = rpool.tile([128, QT], f32, tag="rcs", name="rcs")
                    nc.sync.dma_start(rcs[0:96, :], rc[32:128, :])
                    on = rpool.tile([128, QT], bft, tag="on", name="on")
                    nc.vector.tensor_mul(
                        on[0:96, :], pp[0:96, 0:QT], rcs[0:96, :]
                    )
                    nc.sync.dma_start(
                        oT[g // 2][
                            64 * (g % 2) : 64 * (g % 2) + 32,
                            QT * qt : QT * qt + QT,
                        ],
                        on[0:32, :],
                    )
                    nc.sync.dma_start(
                        oT[g // 2][
                            64 * (g % 2) + 32 : 64 * (g % 2) + 64,
                            QT * qt : QT * qt + QT,
                        ],
                        on[64:96, :],
                    )
            def qt_proj(qt):
                # projections for this qtile's columns, in halves of 192 so
                # the second half's conv/store overlaps the first's.
                HQ = QT // 2
                for half in range(2):
                    n0 = QT * qt + HQ * half
                    for oc in range(2):
                        ps = psB.tile([128, 512], f32, tag="ps", name="ps")
                        for cc in range(2):
                            nc.tensor.matmul(
                                ps[:, :HQ],
                                lhsT=wo[cc][:, 128 * oc : 128 * oc + 128],
                                rhs=oT[cc][:, n0 : n0 + HQ],
                                start=(cc == 0),
                                stop=(cc == 1),
                            )
                        nc.vector.tensor_add(
                            res[oc][:, n0 : n0 + HQ],
                            ps[:, :HQ],
                            xres[oc][:, n0 : n0 + HQ],
                        )
                    for oc in range(2):
                        ps = psB.tile([128, 512], f32, tag="ps", name="ps")
                        for cc in range(2):
                            nc.tensor.matmul(
                                ps[:, :HQ],
                                lhsT=wc[cc][:, 128 * oc : 128 * oc + 128],
                                rhs=res[cc][:, n0 : n0 + HQ],
                                start=(cc == 0),
                                stop=(cc == 1),
                            )
                        ob = opool.tile([128, QT], f32, tag="ob", name="ob")
                        nc.vector.tensor_scalar(
                            ob[:, :HQ],
                            ps[:, :HQ],
                            bcr[:, oc : oc + 1],
                            0.0,
                            OP.add,
                            OP.max,
                        )
                        nc.sync.dma_start(
                            out_d[128 * oc : 128 * oc + 128, n0 : n0 + HQ],
                            ob[:, :HQ],
                        )

            def attn_pair(qt, g):
                pp = pair_open()
                for op_i in range(len(_pairs(qt))):
                    attn_op(qt, g, op_i, pp)
                pair_fin(qt, g, pp)

            q_proj(0)
            k_proj(0)
            for i in range(6):
                v_proj(i)
            attn_pair(0, 0)
            attn_pair(0, 1)
            q_proj(1)
            k_proj(1)
            for i in range(6, 12):
                v_proj(i)
            attn_pair(0, 2)
            attn_pair(0, 3)
            qt_proj(0)
            for qt in (1, 2):
                for g in range(4):
                    attn_pair(qt, g)
                qt_proj(qt)

    nc.compile()
    return nc


def _get_program():
    global _PROG
    if _PROG is None:
        _PROG = _build_program()
    return _PROG


def _prep_core_inputs(core, x, in_proj_w, in_proj_b, out_w, out_b, conv_w, conv_b):
    b, half = core // 2, core % 2
    ximg = x[b].reshape(C, H, W)
    if half == 1:
        ximg = ximg[:, ::-1, :]  # row-flip: half-1 becomes half-0 geometry
    bv = in_proj_b[2 * C :].astype(np.float32)
    rbias = out_b.astype(np.float32) + out_w.astype(np.float32) @ bv
    xres = (ximg[:, :ROWS_HALF, :].reshape(C, NQ) + rbias[:, None]).astype(
        np.float32
    )
    # chunk-major band: [C, band(4), strip(3), r(8), c(16)]
    xcm = (
        ximg[:, :BAND_ROWS, :]
        .reshape(C, 4, 8, 3, 16)
        .transpose(0, 1, 3, 2, 4)
        .reshape(C, NKCM)
    )
    return {
        "xqT": np.ascontiguousarray(
            ximg[:, :ROWS_HALF, :].reshape(C, NQ)
        ).astype(bf16),
        "xcm": np.ascontiguousarray(xcm).astype(bf16),
        "xres": xres,
        "wqkT": np.ascontiguousarray(in_proj_w[: 2 * C].T).astype(bf16),
        "wvT": np.ascontiguousarray(in_proj_w[2 * C :].T).astype(bf16),
        "woT": np.ascontiguousarray(out_w.T).astype(bf16),
        "wcT": np.ascontiguousarray(conv_w.T).astype(bf16),
        "bq": np.ascontiguousarray(
            in_proj_b[:C].reshape(2, 128).T
        ).astype(np.float32),
        "bcrep": np.ascontiguousarray(conv_b.reshape(2, 128).T).astype(
            np.float32
        ),
        "masks": _masks(),
    }


_MASK_CACHE = {}


def _masks() -> np.ndarray:
    """[128, MTOT] binary pair-masks, shared by every core.

    Column layout matches MOFF: kind 0 (qt0, band deltas 0/+8) then kind 1
    (qt1/qt2, deltas -8/0/+8); each op concatenates its chunk-pair's
    [128, 8*ncs] packed masks. Key partition p = 16*rk + ck.
    """
    if "m" in _MASK_CACHE:
        return _MASK_CACHE["m"]
    cols = []
    for kind, deltas in ((0, (0, 8)), (1, (-8, 0, 8))):
        ch = [(d, s) for d in deltas for s in range(3)]
        for grp in [ch[i : i + 2] for i in range(0, len(ch), 2)]:
            for d, s in grp:
                ncs = NCS[s]
                rk = np.arange(8)
                ck = np.arange(16)
                rq = np.arange(8)
                cq = CLO[s] + np.arange(ncs)
                row_ok = (
                    np.abs((d + rk)[:, None, None, None] - rq[None, None, :, None])
                    <= HALF
                )
                col_ok = (
                    np.abs(
                        (16 * s + ck)[None, :, None, None]
                        - cq[None, None, None, :]
                    )
                    <= HALF
                )
                m = (row_ok & col_ok).reshape(128, 8 * ncs)
                cols.append(m)
    res = np.concatenate(cols, axis=1).astype(bf16)
    assert res.shape == (128, MTOT)
    _MASK_CACHE["m"] = res
    return res


def kernel(**inputs):
    from concourse.bass_utils import run_bass_kernel_spmd

    args = {k: np.asarray(v) for k, v in inputs.items()}
    nc = _get_program()
    in_maps = [
        _prep_core_inputs(core, **args) for core in range(NCORES)
    ]
    res = run_bass_kernel_spmd(nc, in_maps, core_ids=list(range(NCORES)))
    out = np.zeros((B, C, H, W), np.float32)
    for core in range(NCORES):
        b, half = core // 2, core % 2
        o = res.results[core]["out"].reshape(C, ROWS_HALF, W)
        if half == 1:
            o = o[:, ::-1, :]  # undo the row flip
            out[b][:, ROWS_HALF:, :] = o
        else:
            out[b][:, :ROWS_HALF, :] = o
    return out


# revision 19
# speedup vs baseline: 1.0173x; 1.0173x over previous
"""Trainium2 Bass kernel for ChunkedLocalSelfAttention.

Module: x[B,C,H,W] -> qkv proj -> 8-head local-window attention (17x17
spatial window) -> out proj -> +residual -> 1x1 conv -> relu.
B,C,H,W = 4,256,48,48; N = 2304 tokens per image; head dim 32.

Sharding: 8 cores = 4 batch images x 2 query-row-halves (24 rows each).
Attention output rows only depend on +-8 image rows, so cores need no
communication; the row halo is covered by a 32-row k/v band.

On-core design (scores kept TRANSPOSED: keys on partitions, queries free):
  - 2D key chunking: key chunks are [8 rows x 16 cols] = 128 keys, so the
    score block for a chunk only spans the queries within the +-8 COLUMN
    window of its strip ([8 q-rows x <=32 q-cols] instead of all 48 cols).
    This cuts exp/mask/score/PV element count ~30% vs row-major chunks.
  - k and v are projected from a chunk-major-permuted copy of x (xcm) so
    each chunk's 128 keys are contiguous (matmul stationary APs must have
    a single free dim); q is projected from a row-major copy.
  - k-bias is dropped entirely (constant-per-query terms cancel in
    softmax); q-bias is added during the PSUM->SBUF cast; v-bias is folded
    into the residual on the host (softmax weights sum to 1).
  - score chunks are processed in PAIRS sharing one PSUM bank per head
    (packed 2D crops), halving Activation-engine op count: exp runs on
    [128, 2 heads, E_pair<=448] per op.
  - binary window mask applied multiplicatively AFTER exp via
    scalar_tensor_tensor (TensorScalarPtr hits the DVE 4x perf mode on
    bf16/SBUF; plain TensorTensor only gets 2x).
  - PV += v_chunk.T @ masked per chunk with [v|ones] packed lhsT (M=64);
    the ones columns replicate each head's softmax denominator.
  - normalize: reciprocal (DVE), 32-partition shift (DMA), multiply on
    GpSimd (idle engine), compact to channel order (DMA).
  - out proj, +residual(+out_b+Wo@bv folded on host), 1x1 conv, relu.
"""

import sys

for _p in ("/opt/trn_rl_repo",):
    if _p not in sys.path:
        sys.path.insert(0, _p)

import math

import ml_dtypes
import numpy as np

B, C, H, W = 4, 256, 48, 48
N = H * W
HEADS, HD, HALF = 8, 32, 8
NCORES = 8
ROWS_HALF = H // 2          # 24 query rows per core
NQ = ROWS_HALF * W          # 1152 queries per core
BAND_ROWS = 32              # k/v row band per core (24 + 8 halo)
NKCM = BAND_ROWS * W        # 1536 band tokens, chunk-major
QT = 384                    # queries per tile (8 image rows)
SCALE = 1.0 / math.sqrt(HD)

# 2D chunk geometry: chunks are [8 rows x 16 cols]; strips at cols 0/16/32.
CLO = (0, 8, 24)            # query-col crop per strip
CHI = (24, 40, 48)
NCS = (24, 32, 24)
ES = tuple(8 * n for n in NCS)   # 192, 256, 192

bf16 = ml_dtypes.bfloat16

_PROG = None


def _chunks(qt):
    """[(chunk_id, strip)] for query tile qt; chunk_id indexes the 12
    [8x16] chunks of the 32-row band (band*3 + strip)."""
    nb = 2 if qt == 0 else 3
    boff = (0, 0, 1)[qt]
    return [((br + boff) * 3 + s, s) for br in range(nb) for s in range(3)]


def _pairs(qt):
    ch = _chunks(qt)
    return [ch[i : i + 2] for i in range(0, len(ch), 2)]


def _deltas(qt):
    return (0, 8) if qt == 0 else (-8, 0, 8)


def _mask_offsets():
    """Column offsets of each (kind, op) pair-mask in the packed table."""
    offs = {}
    col = 0
    for kind, deltas in ((0, (0, 8)), (1, (-8, 0, 8))):
        ch = [(d, s) for d in deltas for s in range(3)]
        grps = [ch[i : i + 2] for i in range(0, len(ch), 2)]
        for op_i, grp in enumerate(grps):
            offs[(kind, op_i)] = col
            col += sum(ES[s] for _, s in grp)
    return offs, col


MOFF, MTOT = _mask_offsets()


def _build_program():
    import concourse.bass as bass
    import concourse.mybir as mybir
    import concourse.tile as tile
    from concourse import bacc
    from concourse.alu_op_type import AluOpType as OP

    f32 = mybir.dt.float32
    bft = mybir.dt.bfloat16
    AF = mybir.ActivationFunctionType

    nc = bacc.Bacc(
        "TRN2", target_bir_lowering=False, debug=False, num_devices=NCORES
    )

    def din(name, shape, dt=bft):
        return nc.dram_tensor(name, shape, dt, kind="ExternalInput").ap()

    xq_d = din("xqT", [C, NQ])            # row-major rows [0,24)
    xcm_d = din("xcm", [C, NKCM])         # chunk-major rows [0,32)
    xres_d = din("xres", [C, NQ], f32)
    wqk_d = din("wqkT", [C, 2 * C])
    wv_d = din("wvT", [C, C])
    wo_d = din("woT", [C, C])
    wc_d = din("wcT", [C, C])
    bq_d = din("bq", [128, 2], f32)
    bc_d = din("bcrep", [128, 2], f32)
    mask_d = din("masks", [128, MTOT])
    out_d = nc.dram_tensor("out", [C, NQ], f32, kind="ExternalOutput").ap()

    # SPMD trick: one program serves both row-halves. The host ships half-1
    # images VERTICALLY FLIPPED (attention is equivariant under a row flip),
    # so every core sees half-0 geometry: query rows [0,24), band [0,32).

    with tile.TileContext(nc) as tc:
        import contextlib

        ctx = contextlib.ExitStack()
        with ctx:
            cpool = ctx.enter_context(tc.tile_pool(name="const", bufs=1))
            qkpool = ctx.enter_context(tc.tile_pool(name="qk", bufs=1))
            vpool = ctx.enter_context(tc.tile_pool(name="v", bufs=1))
            epool = ctx.enter_context(tc.tile_pool(name="exp", bufs=4))
            apool = ctx.enter_context(tc.tile_pool(name="attn", bufs=4))
            rpool = ctx.enter_context(tc.tile_pool(name="recip", bufs=3))
            opool = ctx.enter_context(tc.tile_pool(name="outb", bufs=3))
            psA = ctx.enter_context(
                tc.tile_pool(name="psA", bufs=2, space="PSUM")
            )
            psPP = ctx.enter_context(
                tc.tile_pool(name="psPP", bufs=2, space="PSUM")
            )
            psB = ctx.enter_context(
                tc.tile_pool(name="psB", bufs=2, space="PSUM")
            )

            # ---- constants / inputs to SBUF ----
            xq = [cpool.tile([128, NQ], bft, tag=f"xq{t}", name=f"xq{t}") for t in range(2)]
            xcm = [cpool.tile([128, NKCM], bft, tag=f"xcm{t}", name=f"xcm{t}") for t in range(2)]
            wqk = [cpool.tile([128, 2 * C], bft, tag=f"wqk{t}", name=f"wqk{t}") for t in range(2)]
            wv = [cpool.tile([128, C], bft, tag=f"wv{t}", name=f"wv{t}") for t in range(2)]
            wo = [cpool.tile([128, C], bft, tag=f"wo{t}", name=f"wo{t}") for t in range(2)]
            wc = [cpool.tile([128, C], bft, tag=f"wc{t}", name=f"wc{t}") for t in range(2)]
            bq = cpool.tile([128, 2], f32, tag="bq")
            bcr = cpool.tile([128, 2], f32, tag="bcr")
            zrow = cpool.tile([1, 512], bft, tag="zrow")
            nc.vector.memset(zrow[:], 0.0)
            msk = cpool.tile([128, MTOT], bft, tag="msk")
            xres = [cpool.tile([128, NQ], f32, tag=f"xres{t}", name=f"xres{t}") for t in range(2)]
            # preload the Exp table while DMAs are in flight
            actwarm = cpool.tile([1, 8], bft, tag="actwarm")
            nc.scalar.activation(actwarm[:], zrow[0:1, 0:8], AF.Exp)
            # load order = first-use order: q path, then k/v, masks, tail
            def _ld(t, dst, src):
                nc.sync.dma_start(dst[t][:], src[128 * t : 128 * t + 128, :])
            for t in range(2):
                _ld(t, xq, xq_d)
                _ld(t, wqk, wqk_d)
            nc.sync.dma_start(bq[:], bq_d[:])
            for t in range(2):
                _ld(t, xcm, xcm_d)
                _ld(t, wv, wv_d)
            nc.sync.dma_start(msk[:], mask_d[:])
            for t in range(2):
                _ld(t, wo, wo_d)
                _ld(t, wc, wc_d)
                _ld(t, xres, xres_d)
            nc.sync.dma_start(bcr[:], bc_d[:])

            # ---- phase 1: projections ----
            # q tiles [128ch, 1152 row-major]; k tiles [128ch, 1536 chunk-major]
            qkq = [qkpool.tile([128, NQ], bft, tag=f"qkq{i}", name=f"qkq{i}") for i in range(2)]
            qkk = [qkpool.tile([128, NKCM], bft, tag=f"qkk{i}", name=f"qkk{i}") for i in range(2)]
            # v tiles per chunk: head h cols [64h,64h+32)=v_h, [64h+32,64h+64)=1
            vt = [vpool.tile([128, 8 * 64], bft, tag=f"v{i}", name=f"v{i}") for i in range(12)]
            for i in range(12):
                va = vt[i][:].rearrange("p (h two v) -> p h two v", two=2, v=32)
                nc.gpsimd.memset(va[:, :, 1, :], 1.0)

            def q_proj(qc, tiles=(0, 384, 768)):
                for n0 in tiles:
                    ps = psB.tile([128, 512], f32, tag="ps", name="ps")
                    for cc in range(2):
                        nc.tensor.matmul(
                            ps[:, :QT],
                            lhsT=wqk[cc][:, 128 * qc : 128 * qc + 128],
                            rhs=xq[cc][:, n0 : n0 + QT],
                            start=(cc == 0),
                            stop=(cc == 1),
                        )
                    nc.vector.tensor_scalar_add(
                        qkq[qc][:, n0 : n0 + QT], ps[:, :QT], bq[:, qc : qc + 1]
                    )

            def k_proj(kc, tiles=(0, 512, 1024)):
                for n0 in tiles:
                    ps = psB.tile([128, 512], f32, tag="ps", name="ps")
                    for cc in range(2):
                        nc.tensor.matmul(
                            ps[:],
                            lhsT=wqk[cc][:, 256 + 128 * kc : 256 + 128 * kc + 128],
                            rhs=xcm[cc][:, n0 : n0 + 512],
                            start=(cc == 0),
                            stop=(cc == 1),
                        )
                    nc.vector.tensor_copy(qkk[kc][:, n0 : n0 + 512], ps[:])

            def v_proj(i):
                n0 = 128 * i
                ps = psB.tile([128, 512], f32, tag="ps", name="ps")
                for cc in range(2):
                    nc.tensor.matmul(
                        ps[:, :C],
                        lhsT=xcm[cc][:, n0 : n0 + 128],
                        rhs=wv[cc][:],
                        start=(cc == 0),
                        stop=(cc == 1),
                    )
                va = vt[i][:].rearrange("p (h two v) -> p h two v", two=2, v=32)
                nc.scalar.copy(
                    va[:, :, 0, :],
                    ps[:, :C].rearrange("p (h v) -> p h v", v=32),
                )

            # ---- phase 2: attention ----
            oT = [cpool.tile([128, NQ], bft, tag=f"oT{g}", name=f"oT{g}") for g in range(2)]
            res = [cpool.tile([128, NQ], bft, tag=f"res{t}", name=f"res{t}") for t in range(2)]

            def pair_open():
                # pp rows: [pv_a(0:32)|S_a(32:64)|pv_b(64:96)|S_b(96:128)]
                # zero-matmul opens the bank: clears pending-zero over the
                # full [0:384] so per-chunk 2D-crop PVs can accumulate.
                pp = psPP.tile([128, 512], f32, tag="pp", name="pp")
                nc.tensor.matmul(
                    pp[:, 0:QT],
                    lhsT=zrow[:, 0:128],
                    rhs=zrow[:, 0:QT],
                    start=True,
                    stop=False,
                    skip_group_check=True,
                )
                return pp

            def attn_op(qt, g, op_i, pp):
                q0row = 8 * qt
                kind = 0 if qt == 0 else 1
                ops = _pairs(qt)
                if True:
                    for grp in [ops[op_i]]:
                        sc = psA.tile([128, 1024], f32, tag="sc", name="sc")
                        offs = []
                        off = 0
                        for ck_id, s in grp:
                            ncs = NCS[s]
                            for hh in range(2):
                                h = 2 * g + hh
                                qc, krow = h // 4, 32 * (h % 4)
                                out_v = sc[
                                    :, 512 * hh + off : 512 * hh + off + 8 * ncs
                                ].rearrange("p (r c) -> p r c", c=ncs)
                                rhs = qkq[qc][krow : krow + 32, :].rearrange(
                                    "p (r c) -> p r c", c=48
                                )[:, q0row : q0row + 8, CLO[s] : CHI[s]]
                                nc.tensor.matmul(
                                    out_v,
                                    lhsT=qkk[qc][
                                        krow : krow + 32,
                                        128 * ck_id : 128 * ck_id + 128,
                                    ],
                                    rhs=rhs,
                                    start=True,
                                    stop=True,
                                    tile_position=(krow, 0),
                                )
                            offs.append(off)
                            off += 8 * ncs
                        ep = off
                        ex = epool.tile([128, 1024], bft, tag="ex", name="ex")
                        sc_v = sc[:].rearrange("p (h q) -> p h q", q=512)[
                            :, :, 0:ep
                        ]
                        ex_v = ex[:, 0 : 2 * ep].rearrange(
                            "p (h q) -> p h q", q=ep
                        )
                        nc.scalar.activation(ex_v, sc_v, AF.Exp, scale=SCALE)
                        ma = apool.tile([128, 1024], bft, tag="ma", name="ma")
                        ma_v = ma[:, 0 : 2 * ep].rearrange(
                            "p (h q) -> p h q", q=ep
                        )
                        mk = msk[
                            :, MOFF[(kind, op_i)] : MOFF[(kind, op_i)] + ep
                        ]
                        # TensorTensor gets the DVE 2x_1p mode (bf16, packed);
                        # a few ops go to the otherwise-idle GpSimd engine.
                        mask_eng = nc.vector
                        mask_eng.tensor_mul(
                            ma_v,
                            ex_v,
                            mk[:, None, :].broadcast_to([128, 2, ep]),
                        )
                        last_op = op_i == len(ops) - 1
                        for j, (ck_id, s) in enumerate(grp):
                            ncs = NCS[s]
                            vi = vt[ck_id]
                            for hh in range(2):
                                h = 2 * g + hh
                                out_v = pp[
                                    64 * hh : 64 * hh + 64, 0:QT
                                ].rearrange("p (r c) -> p r c", c=48)[
                                    :, :, CLO[s] : CHI[s]
                                ]
                                nc.tensor.matmul(
                                    out_v,
                                    lhsT=vi[:, 64 * h : 64 * h + 64],
                                    rhs=ma[
                                        :,
                                        ep * hh + offs[j] : ep * hh + offs[j] + 8 * ncs,
                                    ],
                                    start=False,
                                    stop=(
                                        last_op
                                        and j == len(grp) - 1
                                        and hh == 1
                                    ),
                                    skip_group_check=True,
                                    tile_position=(0, 64 * hh),
                                )

            def pair_fin(qt, g, pp):
                if True:
                    # normalize: recip sums, shift down 32 partitions onto pv
                    # lanes, multiply, compact to channel order.
                    rc = rpool.tile([128, QT], f32, tag="rc", name="rc")
                    nc.vector.reciprocal(rc[:], pp[:, 0:QT])
                    rcs = rpool.tile([128, QT], f32, tag="rcs", name="rcs")
                    nc.sync.dma_start(rcs[0:96, :], rc[32:128, :])
                    on = rpool.tile([128, QT], bft, tag="on", name="on")
                    nc.vector.tensor_mul(
                        on[0:96, :], pp[0:96, 0:QT], rcs[0:96, :]
                    )
                    nc.sync.dma_start(
                        oT[g // 2][
                            64 * (g % 2) : 64 * (g % 2) + 32,
                            QT * qt : QT * qt + QT,
                        ],
                        on[0:32, :],
                    )
                    nc.sync.dma_start(
                        oT[g // 2][
                            64 * (g % 2) + 32 : 64 * (g % 2) + 64,
                            QT * qt : QT * qt + QT,
                        ],
                        on[64:96, :],
                    )
            def qt_proj(qt):
                # projections for this qtile's columns, in halves of 192 so
                # the second half's conv/store overlaps the first's.
                HQ = QT // 2
                for half in range(2):
                    n0 = QT * qt + HQ * half
                    for oc in range(2):
                        ps = psB.tile([128, 512], f32, tag="ps", name="ps")
                        for cc in range(2):
                            nc.tensor.matmul(
                                ps[:, :HQ],
                                lhsT=wo[cc][:, 128 * oc : 128 * oc + 128],
                                rhs=oT[cc][:, n0 : n0 + HQ],
                                start=(cc == 0),
                                stop=(cc == 1),
                            )
                        nc.vector.tensor_add(
                            res[oc][:, n0 : n0 + HQ],
                            ps[:, :HQ],
                            xres[oc][:, n0 : n0 + HQ],
                        )
                    for oc in range(2):
                        ps = psB.tile([128, 512], f32, tag="ps", name="ps")
                        for cc in range(2):
                            nc.tensor.matmul(
                                ps[:, :HQ],
                                lhsT=wc[cc][:, 128 * oc : 128 * oc + 128],
                                rhs=res[cc][:, n0 : n0 + HQ],
                                start=(cc == 0),
                                stop=(cc == 1),
                            )
                        ob = opool.tile([128, QT], f32, tag="ob", name="ob")
                        nc.vector.tensor_scalar(
                            ob[:, :HQ],
                            ps[:, :HQ],
                            bcr[:, oc : oc + 1],
                            0.0,
                            OP.add,
                            OP.max,
                        )
                        nc.sync.dma_start(
                            out_d[128 * oc : 128 * oc + 128, n0 : n0 + HQ],
                            ob[:, :HQ],
                        )

            def attn_pair(qt, g):
                pp = pair_open()
                for op_i in range(len(_pairs(qt))):
                    attn_op(qt, g, op_i, pp)
                pair_fin(qt, g, pp)

            # Fine-grained startup interleave: qt0 pair g0 only needs q tile
            # 0 of qc0, k chunks 0-5 and v chunks 0-5; feed its ops as soon
            # as their chunks are projected so the Activation engine starts
            # early, and defer the rest of phase 1 behind qt0's attention.
            q_proj(0, tiles=(0,))
            k_proj(0, tiles=(0,))
            v_proj(0)
            v_proj(1)
            pp00 = pair_open()
            attn_op(0, 0, 0, pp00)
            v_proj(2)
            v_proj(3)
            attn_op(0, 0, 1, pp00)
            k_proj(0, tiles=(512,))
            v_proj(4)
            v_proj(5)
            attn_op(0, 0, 2, pp00)
            pair_fin(0, 0, pp00)
            attn_pair(0, 1)
            q_proj(1, tiles=(0,))
            k_proj(1, tiles=(0, 512))
            attn_pair(0, 2)
            attn_pair(0, 3)
            q_proj(0, tiles=(384, 768))
            k_proj(0, tiles=(1024,))
            q_proj(1, tiles=(384, 768))
            k_proj(1, tiles=(1024,))
            for i in range(6, 12):
                v_proj(i)
            qt_proj(0)
            for qt in (1, 2):
                for g in range(4):
                    attn_pair(qt, g)
                qt_proj(qt)

    nc.compile()
    return nc


def _get_program():
    global _PROG
    if _PROG is None:
        _PROG = _build_program()
    return _PROG


def _prep_core_inputs(core, x, in_proj_w, in_proj_b, out_w, out_b, conv_w, conv_b):
    b, half = core // 2, core % 2
    ximg = x[b].reshape(C, H, W)
    if half == 1:
        ximg = ximg[:, ::-1, :]  # row-flip: half-1 becomes half-0 geometry
    bv = in_proj_b[2 * C :].astype(np.float32)
    rbias = out_b.astype(np.float32) + out_w.astype(np.float32) @ bv
    xres = (ximg[:, :ROWS_HALF, :].reshape(C, NQ) + rbias[:, None]).astype(
        np.float32
    )
    # chunk-major band: [C, band(4), strip(3), r(8), c(16)]
    xcm = (
        ximg[:, :BAND_ROWS, :]
        .reshape(C, 4, 8, 3, 16)
        .transpose(0, 1, 3, 2, 4)
        .reshape(C, NKCM)
    )
    return {
        "xqT": np.ascontiguousarray(
            ximg[:, :ROWS_HALF, :].reshape(C, NQ)
        ).astype(bf16),
        "xcm": np.ascontiguousarray(xcm).astype(bf16),
        "xres": xres,
        "wqkT": np.ascontiguousarray(in_proj_w[: 2 * C].T).astype(bf16),
        "wvT": np.ascontiguousarray(in_proj_w[2 * C :].T).astype(bf16),
        "woT": np.ascontiguousarray(out_w.T).astype(bf16),
        "wcT": np.ascontiguousarray(conv_w.T).astype(bf16),
        "bq": np.ascontiguousarray(
            in_proj_b[:C].reshape(2, 128).T
        ).astype(np.float32),
        "bcrep": np.ascontiguousarray(conv_b.reshape(2, 128).T).astype(
            np.float32
        ),
        "masks": _masks(),
    }


_MASK_CACHE = {}


def _masks() -> np.ndarray:
    """[128, MTOT] binary pair-masks, shared by every core.

    Column layout matches MOFF: kind 0 (qt0, band deltas 0/+8) then kind 1
    (qt1/qt2, deltas -8/0/+8); each op concatenates its chunk-pair's
    [128, 8*ncs] packed masks. Key partition p = 16*rk + ck.
    """
    if "m" in _MASK_CACHE:
        return _MASK_CACHE["m"]
    cols = []
    for kind, deltas in ((0, (0, 8)), (1, (-8, 0, 8))):
        ch = [(d, s) for d in deltas for s in range(3)]
        for grp in [ch[i : i + 2] for i in range(0, len(ch), 2)]:
            for d, s in grp:
                ncs = NCS[s]
                rk = np.arange(8)
                ck = np.arange(16)
                rq = np.arange(8)
                cq = CLO[s] + np.arange(ncs)
                row_ok = (
                    np.abs((d + rk)[:, None, None, None] - rq[None, None, :, None])
                    <= HALF
                )
                col_ok = (
                    np.abs(
                        (16 * s + ck)[None, :, None, None]
                        - cq[None, None, None, :]
                    )
                    <= HALF
                )
                m = (row_ok & col_ok).reshape(128, 8 * ncs)
                cols.append(m)
    res = np.concatenate(cols, axis=1).astype(bf16)
    assert res.shape == (128, MTOT)
    _MASK_CACHE["m"] = res
    return res


def kernel(**inputs):
    from concourse.bass_utils import run_bass_kernel_spmd

    args = {k: np.asarray(v) for k, v in inputs.items()}
    nc = _get_program()
    in_maps = [
        _prep_core_inputs(core, **args) for core in range(NCORES)
    ]
    res = run_bass_kernel_spmd(nc, in_maps, core_ids=list(range(NCORES)))
    out = np.zeros((B, C, H, W), np.float32)
    for core in range(NCORES):
        b, half = core // 2, core % 2
        o = res.results[core]["out"].reshape(C, ROWS_HALF, W)
        if half == 1:
            o = o[:, ::-1, :]  # undo the row flip
            out[b][:, ROWS_HALF:, :] = o
        else:
            out[b][:, :ROWS_HALF, :] = o
    return out
